# revision 9
# baseline (speedup 1.0000x reference)
"""Trainium2 Bass kernel for nn_AttentionEncoderToFixedLength.

2-layer transformer encoder (D=512, H=8 heads, T=1024) with attentive pooling.
Sharding: data-parallel over batch B=8 across the 8 NeuronCores (1 batch row
per core); weights replicated. Everything on-chip per core:

- Activations are kept feature-major [D, T] in SBUF so every linear contracts
  over the partition dim; weights are host-transposed (and the per-head QKV
  split matmuls are host-fused into the preceding MLP's second linear).
- Attention scores are computed key-major [s, t] so the padding mask folds
  into the per-partition bias of the single Exp pass, and softmax sums come
  from an extra all-ones column appended to V (no extra passes over T^2).
- Softmax normalization is applied via a K=1 ones-matmul broadcast of 1/sum.
- LayerNorm over the feature (partition) dim uses ones-vector matmuls for the
  stats and K=1/K=2 broadcast matmuls for the normalization coefficients.

Compute dtype: bf16 operands with f32 PSUM accumulation; the small broadcast /
stats operands use float32r (TF32-like, full PE rate).
"""

import sys
import types

import numpy as np
import ml_dtypes

import concourse.bacc as bacc
import concourse.mybir as mybir
import concourse.tile as tile
from concourse.bass_utils import run_bass_kernel_spmd
from concourse.masks import make_identity

F32 = mybir.dt.float32
F32R = mybir.dt.float32r
BF16 = mybir.dt.bfloat16
AF = mybir.ActivationFunctionType
OP = mybir.AluOpType
BF_NP = ml_dtypes.bfloat16

B, T, IN = 8, 1024, 80
D, M, H, L = 512, 512, 8, 2
E = D // H           # 64
EP1 = E + 1          # 65 (per-head V width incl. ones column)
P = 128
TS = 512             # t-chunk (matmul moving free dim)
NCH = T // TS        # 2
NSUB = D // P        # 4
NEG = -1e4           # masked-out additive logit

BIAS_NAMES = []
for _l in range(L):
    BIAS_NAMES += [f"L{_l}_{n}" for n in
                   ("b1", "b2", "qb1", "kb1", "vb1", "qbeff", "kbeff",
                    "h2ab", "fb1", "fb2", "lng")]
BIAS_NAMES += ["ffb1", "fwb1", "blast", "blogit"]
BIAS_IDX = {n: i for i, n in enumerate(BIAS_NAMES)}
NB = len(BIAS_NAMES)


# ---------------------------------------------------------------- host prep --

def _np(v):
    return np.asarray(v, dtype=np.float32)


def _bf(a):
    return np.ascontiguousarray(np.asarray(a, np.float32)).astype(BF_NP)


def host_weights(params):
    """Shared (core-independent) DRAM tensors: transposed/fused weights."""
    W = {}
    biases = np.zeros((NB, 512), np.float32)

    def set_bias(name, vec):
        v = _np(vec)
        biases[BIAS_IDX[name], : v.shape[0]] = v

    inv = (10.0 ** (4.0 * np.arange(0.0, 1.0, 2.0 / D, dtype=np.float32))).astype(np.float32)
    ang = np.arange(T, dtype=np.float32)[:, None] / inv[None, :]
    pe = np.stack([np.sin(ang), np.cos(ang)], -1).reshape(T, D).astype(np.float32)
    W["pe"] = _bf(pe.T)                                     # [D, T]

    for l, p in enumerate(params["layers"]):
        pre = f"L{l}_"
        W[f"w1t{l}"] = _bf(_np(p["to_hidden"]["l1"]["w"]).T)      # [i, M]
        set_bias(pre + "b1", p["to_hidden"]["l1"]["b"])
        W[f"w2t{l}"] = _bf(_np(p["to_hidden"]["l2"]["w"]).T)      # [M, D]
        set_bias(pre + "b2", p["to_hidden"]["l2"]["b"])
        effs = {}
        for n in ("q", "k", "v"):
            mp, sp = p[n + "_mlp"], p[n + "_split"]
            W[f"{n}1t{l}"] = _bf(_np(mp["l1"]["w"]).T)            # [D, M]
            set_bias(pre + n + "b1", mp["l1"]["b"])
            ws = _np(sp["w"]).reshape(H * E, D)
            effs[n] = (ws @ _np(mp["l2"]["w"]),                   # [512, M]
                       ws @ _np(mp["l2"]["b"]) + _np(sp["b"]).reshape(-1))
        qw, qb = effs["q"]
        s = 1.0 / np.sqrt(np.float32(E))
        W[f"qeff{l}"] = _bf((qw * s).T)                           # [M, 512]
        set_bias(pre + "qbeff", qb * s)
        kw, kb = effs["k"]
        W[f"keff{l}"] = _bf(kw.T)
        set_bias(pre + "kbeff", kb)
        vw, vb = effs["v"]
        vaug = np.zeros((5 * P, H * EP1), np.float32)             # [640, 520]
        for h in range(H):
            c = h * EP1
            vaug[:M, c : c + E] = vw.T[:, h * E : (h + 1) * E]
            vaug[M, c : c + E] = vb[h * E : (h + 1) * E]
            vaug[M, c + E] = 1.0
        W[f"vaug{l}"] = _bf(vaug)
        W[f"h2at{l}"] = _bf(_np(p["h2a"]["w"]).T)
        set_bias(pre + "h2ab", p["h2a"]["b"])
        W[f"f1t{l}"] = _bf(_np(p["ff"]["l1"]["w"]).T)
        set_bias(pre + "fb1", p["ff"]["l1"]["b"])
        W[f"f2t{l}"] = _bf(_np(p["ff"]["l2"]["w"]).T)
        set_bias(pre + "fb2", p["ff"]["l2"]["b"])
        set_bias(pre + "lng", p["ln_g"])
        gb = np.zeros((2, 512), np.float32)
        gb[0] = _np(p["ln_g"])
        gb[1] = _np(p["ln_b"])
        W[f"gb{l}"] = gb                                          # f32r dram

    W["ff1t"] = _bf(_np(params["ff_mlp"]["l1"]["w"]).T)
    set_bias("ffb1", params["ff_mlp"]["l1"]["b"])
    ws = _np(params["ff_split"]["w"]).reshape(H * E, D)
    fw_eff = ws @ _np(params["ff_mlp"]["l2"]["w"])
    fb_eff = ws @ _np(params["ff_mlp"]["l2"]["b"]) + _np(params["ff_split"]["b"]).reshape(-1)
    feataug = np.zeros((5 * P, H * E), np.float32)
    feataug[:M] = fw_eff.T
    feataug[M] = fb_eff
    W["feataug"] = _bf(feataug)
    W["fw1t"] = _bf(_np(params["fw_mlp"]["l1"]["w"]).T)
    set_bias("fwb1", params["fw_mlp"]["l1"]["b"])
    wsw = _np(params["fw_split"]["w"])[:, 0, :]                   # [H, D]
    W["wlogit"] = _bf((wsw @ _np(params["fw_mlp"]["l2"]["w"])).T)  # [M, H]
    set_bias("blogit", wsw @ _np(params["fw_mlp"]["l2"]["b"]) + _np(params["fw_split"]["b"])[:, 0])
    W["wlast"] = _bf(_np(params["last"]["w"]).T)                  # [HE, D]
    set_bias("blast", params["last"]["b"])
    # reorder to [p, sub, j] so the DMA is a contiguous 1:1 copy
    W["biases"] = np.ascontiguousarray(
        biases.reshape(NB, NSUB, P).transpose(2, 1, 0))           # [128, 4, NB]
    return W


def host_core_inputs(x, lengths, b):
    """Per-core inputs for batch row b."""
    maskvec = np.where(np.arange(T) < int(lengths[b]), 0.0, NEG).astype(np.float32)
    return {
        "x_fm": _bf(np.asarray(x[b], np.float32).T),                   # [IN, T]
        "mask_s": np.ascontiguousarray(maskvec.reshape(8, P).T),       # [128, 8]
        "mask8": _bf(np.broadcast_to(maskvec, (8, T))),
    }


# ---------------------------------------------------------------- bass build --

def build_nc():
    nc = bacc.Bacc(None, target_bir_lowering=False)

    dram = {}

    def din(name, shape, dt=BF16):
        dram[name] = nc.dram_tensor(name, list(shape), dt, kind="ExternalInput")
        return dram[name]

    din("x_fm", [IN, T])
    din("mask_s", [P, 8], F32)
    din("mask8", [8, T], BF16)
    din("pe", [D, T])
    din("biases", [P, NSUB, NB], F32)
    for l in range(L):
        din(f"w1t{l}", [IN if l == 0 else D, M])
        din(f"w2t{l}", [M, D])
        for n in ("q", "k", "v"):
            din(f"{n}1t{l}", [D, M])
        din(f"qeff{l}", [M, D])
        din(f"keff{l}", [M, D])
        din(f"vaug{l}", [5 * P, H * EP1])
        din(f"h2at{l}", [D, D])
        din(f"f1t{l}", [D, M])
        din(f"f2t{l}", [M, D])
        din(f"gb{l}", [2, 512], F32R)
    din("ff1t", [D, M])
    din("feataug", [5 * P, H * E])
    din("fw1t", [D, M])
    din("wlogit", [M, H])
    din("wlast", [H * E, D])
    out_d = nc.dram_tensor("out", [D], F32, kind="ExternalOutput")

    def r3(ap, subs):
        """DRAM [(subs*128), n] -> [p, subs, n] view."""
        return ap.rearrange("(s p) n -> p s n", p=P)

    with tile.TileContext(nc) as tc:
        import contextlib
        ctx = contextlib.ExitStack()
        with ctx:
            cp = ctx.enter_context(tc.tile_pool(name="const", bufs=1))
            wp = ctx.enter_context(tc.tile_pool(name="wts", bufs=1))
            ap_ = ctx.enter_context(tc.tile_pool(name="acts", bufs=1))
            sp = ctx.enter_context(tc.tile_pool(name="small", bufs=1))
            psA = ctx.enter_context(tc.tile_pool(name="psA", bufs=1, space="PSUM"))
            psB = ctx.enter_context(tc.tile_pool(name="psB", bufs=1, space="PSUM"))
            psC = ctx.enter_context(tc.tile_pool(name="psC", bufs=1, space="PSUM"))

            # ---- constants ----
            pe_t = cp.tile([P, NSUB, T], BF16, name="pe_t", tag="pe")
            nc.sync.dma_start(out=pe_t[:], in_=r3(dram["pe"][:], NSUB))
            x_t = cp.tile([IN, T], BF16, name="x_t", tag="x")
            nc.sync.dma_start(out=x_t[:], in_=dram["x_fm"][:])
            mask_t = cp.tile([P, 8], F32, name="mask_t", tag="mask")
            nc.sync.dma_start(out=mask_t[:], in_=dram["mask_s"][:])
            mask8_t = cp.tile([8, T], BF16, name="mask8_t", tag="mask8")
            nc.sync.dma_start(out=mask8_t[:], in_=dram["mask8"][:])
            bias_t = cp.tile([P, NSUB, NB], F32, name="bias_t", tag="bias")
            nc.sync.dma_start(out=bias_t[:], in_=dram["biases"][:])
            gb_t = []
            for l in range(L):
                g_row = cp.tile([1, 512], F32R, name=f"g_row{l}", tag=f"g{l}")
                b_row = cp.tile([1, 512], F32R, name=f"b_row{l}", tag=f"b{l}")
                nc.sync.dma_start(out=g_row[:], in_=dram[f"gb{l}"][0:1, :])
                nc.sync.dma_start(out=b_row[:], in_=dram[f"gb{l}"][1:2, :])
                gb_t.append((g_row, b_row))
            ones_k = cp.tile([P, 1], BF16, name="ones_k", tag="ones_k")
            nc.vector.memset(ones_k[:], 1.0)
            ones_f = cp.tile([1, TS], F32, name="ones_f", tag="ones_f")
            nc.vector.memset(ones_f[:], 1.0)
            ones_m = cp.tile([1, P], F32R, name="ones_m", tag="ones_m")
            nc.vector.tensor_copy(ones_m[:], ones_f[:, 0:P])
            ones_row = cp.tile([1, TS], F32R, name="ones_row", tag="ones_row")
            nc.vector.tensor_copy(ones_row[:], ones_f[:])
            ident = cp.tile([P, P], BF16, name="ident", tag="ident")
            make_identity(nc, ident[:])
            out_sb = cp.tile([P, NSUB, 1], F32, name="out_sb", tag="outsb")

            def bias_ap(name, sub):
                return bias_t[:, sub, BIAS_IDX[name] : BIAS_IDX[name] + 1]

            # ---- helpers ----
            def load_w(dname, subs=NSUB, tag="wt", width=None):
                dt_ = dram[dname]
                w = int(dt_.shape[1]) if width is None else width
                if int(dt_.shape[0]) == IN:  # layer-0 l1 weight
                    t_ = wp.tile([IN, 1, w], BF16, name=f"t_{dname}", tag="w1l0", bufs=1)
                    nc.sync.dma_start(out=t_[:, 0, :], in_=dt_[:])
                else:
                    nb = (int(dt_.shape[0]) + P - 1) // P
                    t_ = wp.tile([P, nb, w], BF16, name=f"t_{dname}", tag=tag, bufs=(2 if tag == "vaug" else 4))
                    nc.sync.dma_start(out=t_[:], in_=r3(dt_[:], nb))
                return t_

            def k_parts(tile_, kdim):
                """[(kp, sub), ...] covering contraction dim kdim of tile_."""
                parts = []
                left = kdim
                s = 0
                while left > 0:
                    kp = min(P, left)
                    parts.append((kp, s))
                    left -= kp
                    s += 1
                return parts

            def linear_fm(wt, rhs, kdim, od, evict, rhs_chunk=None):
                """dst[od, T] = wt[kdim, od]^T-contract rhs[kdim, T]; evict(ps, mb, ch, chs)."""
                parts = k_parts(wt, kdim)
                for mb in range(od // P):
                    msl = slice(mb * P, (mb + 1) * P)
                    for ch in range(NCH):
                        chs = slice(ch * TS, (ch + 1) * TS)
                        ps = psA.tile([P, TS], F32, name=f"lps{mb}{ch}", tag="psA", bufs=4)
                        for i, (kp, s) in enumerate(parts):
                            nc.tensor.matmul(
                                ps[:, :],
                                wt[0:kp, s, msl],
                                rhs[0:kp, s, chs] if rhs_chunk is None else rhs_chunk(kp, s, chs),
                                start=(i == 0), stop=(i == len(parts) - 1))
                        evict(ps, mb, ch, chs)

            def act_evict(dst, func, bname):
                def f(ps, mb, ch, chs):
                    nc.scalar.activation(dst[:, mb, chs], ps[:, :], func,
                                         bias=bias_ap(bname, mb), scale=1.0)
                return f

            def stt_evict(dst, bname, res):
                def f(ps, mb, ch, chs):
                    nc.vector.scalar_tensor_tensor(
                        dst[:, mb, chs], ps[:, :], bias_ap(bname, mb), res[:, mb, chs],
                        op0=OP.add, op1=OP.add)
                return f

            def ts_evict(dst, bname):
                def f(ps, mb, ch, chs):
                    nc.vector.tensor_scalar_add(dst[:, mb, chs], ps[:, :], bias_ap(bname, mb))
                return f

            def tok_matmul(dst, lhs_parts, wt, width, dst_f32=False):
                """dst[t, width] (token-major [P, 8, width]) = lhs^T-contract wt."""
                for tb in range(T // P):
                    tsl = slice(tb * P, (tb + 1) * P)
                    nsplit = [(0, min(width, TS))]
                    if width > TS:
                        nsplit.append((TS, width - TS))
                    for (n0, nw) in nsplit:
                        ps = psA.tile([P, TS], F32, name=f"vps{tb}{n0}", tag="psA", bufs=4)
                        for i, (kp, s, lap) in enumerate(lhs_parts):
                            nc.tensor.matmul(
                                ps[:, 0:nw], lap(kp, s, tsl), wt[0:kp, s, n0:n0 + nw],
                                start=(i == 0), stop=(i == len(lhs_parts) - 1))
                        nc.vector.tensor_copy(dst[:, tb, n0:n0 + nw], ps[:, 0:nw])

            # ---- encoder layers ----
            h_cur = None
            for l in range(L):
                hid = ap_.tile([P, NSUB, T], BF16, name=f"hid{l}", tag="hid4", bufs=2)
                w1 = load_w(f"w1t{l}")
                if l == 0:
                    linear_fm(w1, x_t.rearrange("p (o n) -> p o n", o=1), IN, M,
                              act_evict(hid, AF.Tanh, f"L{l}_b1"))
                else:
                    linear_fm(w1, h_cur, D, M, act_evict(hid, AF.Tanh, f"L{l}_b1"))

                h_new = ap_.tile([P, NSUB, T], BF16, name=f"h{l}", tag="h", bufs=2)
                w2 = load_w(f"w2t{l}")

                def pe_evict(ps, mb, ch, chs):
                    nc.vector.scalar_tensor_tensor(
                        h_new[:, mb, chs], ps[:, :], bias_ap(f"L{l}_b2", mb),
                        pe_t[:, mb, chs], op0=OP.add, op1=OP.add)
                linear_fm(w2, hid, M, D, pe_evict)
                h_cur = h_new

                # qkv projections (fused l2+split)
                a_q = ap_.tile([P, NSUB, T], BF16, name=f"aq{l}", tag="hid4", bufs=2)
                linear_fm(load_w(f"q1t{l}"), h_cur, D, M, act_evict(a_q, AF.Tanh, f"L{l}_qb1"))
                q_fm = ap_.tile([P, NSUB, T], BF16, name=f"qfm{l}", tag="qk", bufs=2)
                linear_fm(load_w(f"qeff{l}"), a_q, M, D, ts_evict(q_fm, f"L{l}_qbeff"))

                a_k = ap_.tile([P, NSUB, T], BF16, name=f"ak{l}", tag="hid4", bufs=2)
                linear_fm(load_w(f"k1t{l}"), h_cur, D, M, act_evict(a_k, AF.Tanh, f"L{l}_kb1"))
                k_fm = ap_.tile([P, NSUB, T], BF16, name=f"kfm{l}", tag="qk", bufs=2)
                linear_fm(load_w(f"keff{l}"), a_k, M, D, ts_evict(k_fm, f"L{l}_kbeff"))

                a_v = ap_.tile([P, 5, T], BF16, name=f"av{l}", tag="hid5", bufs=1)
                linear_fm(load_w(f"v1t{l}"), h_cur, D, M, act_evict(a_v, AF.Tanh, f"L{l}_vb1"))
                nc.vector.memset(a_v[0:1, 4, :], 1.0)  # ones row for bias/sum tricks
                v_tok = ap_.tile([P, T // P, H * EP1], BF16, name=f"vtok{l}", tag="vtok", bufs=2)
                vw = load_w(f"vaug{l}", tag="vaug")
                av_parts = [(kp, s, (lambda kp_, s_, tsl: a_v[0:kp_, s_, tsl]))
                            for (kp, s) in (k_parts(a_v, M) + [(1, 4)])]
                tok_matmul(v_tok, av_parts, vw, H * EP1)

                # attention
                att_fm = ap_.tile([P, NSUB, T], BF16, name=f"att{l}", tag="attf", bufs=1)
                for ch in range(NCH):
                    chs = slice(ch * TS, (ch + 1) * TS)
                    for hh in range(H):
                        sub = hh // 2
                        pb = (hh % 2) * E
                        prange = slice(pb, pb + E)
                        ew = ap_.tile([P, T // P, TS], BF16, name=f"ew{l}{ch}{hh}", tag="expw", bufs=2)
                        for sb in range(T // P):
                            ssl = slice(sb * P, (sb + 1) * P)
                            ps_s = psA.tile([P, TS], F32, name=f"sc{sb}", tag="psA", bufs=4)
                            nc.tensor.matmul(ps_s[:, :], k_fm[prange, sub, ssl],
                                             q_fm[prange, sub, chs], start=True, stop=True)
                            nc.scalar.activation(ew[:, sb, :], ps_s[:, :], AF.Exp,
                                                 bias=mask_t[:, sb : sb + 1], scale=1.0)
                        ps_att = psB.tile([P, TS], F32, name="psatt", tag="psB", bufs=2)
                        for sb in range(T // P):
                            nc.tensor.matmul(ps_att[0:EP1, :],
                                             v_tok[:, sb, hh * EP1 : (hh + 1) * EP1],
                                             ew[:, sb, :],
                                             start=(sb == 0), stop=(sb == T // P - 1))
                        rec = sp.tile([1, TS], F32R, name="rec", tag="vec", bufs=6)
                        with nc.allow_low_precision("f32r recip for broadcast matmul"):
                            nc.vector.reciprocal(rec[:], ps_att[E : E + 1, :])
                        ps_rbc = psC.tile([P, TS], F32, name="psrbc", tag="psC", bufs=2)
                        nc.tensor.matmul(ps_rbc[:, :], ones_m[:], rec[:], start=True, stop=True)
                        rbc = sp.tile([P, TS], F32, name="rbc", tag="rbc", bufs=2)
                        nc.scalar.copy(rbc[:, :], ps_rbc[:, :])
                        nc.vector.tensor_tensor(att_fm[prange, sub, chs], ps_att[0:E, :],
                                                rbc[0:E, :], op=OP.mult)

                # h2a + residual -> y ; LN -> h2
                y1 = ap_.tile([P, NSUB, T], BF16, name=f"y1_{l}", tag="ybig", bufs=2)
                linear_fm(load_w(f"h2at{l}"), att_fm, D, D, stt_evict(y1, f"L{l}_h2ab", h_cur))
                h2 = ap_.tile([P, NSUB, T], BF16, name=f"h2_{l}", tag="ybig", bufs=2)
                layer_norm(nc, tc, sp, psB, psC, y1, h2, gb_t[l], bias_ap, f"L{l}_lng",
                           ones_k, ones_m, ones_row, ap_)

                # ff + residual -> y2 ; LN -> h_next
                a_f = ap_.tile([P, NSUB, T], BF16, name=f"af{l}", tag="hid4", bufs=2)
                linear_fm(load_w(f"f1t{l}"), h2, D, M, act_evict(a_f, AF.Relu, f"L{l}_fb1"))
                y2 = ap_.tile([P, NSUB, T], BF16, name=f"y2_{l}", tag="ybig", bufs=2)
                linear_fm(load_w(f"f2t{l}"), a_f, M, D, stt_evict(y2, f"L{l}_fb2", h2))
                h_nx = ap_.tile([P, NSUB, T], BF16, name=f"hn{l}", tag="h", bufs=2)
                layer_norm(nc, tc, sp, psB, psC, y2, h_nx, gb_t[l], bias_ap, f"L{l}_lng",
                           ones_k, ones_m, ones_row, ap_)
                h_cur = h_nx

            # ---- attentive pooling head ----
            a_ff = ap_.tile([P, 5, T], BF16, name="aff", tag="hid5", bufs=1)
            linear_fm(load_w("ff1t"), h_cur, D, M, act_evict(a_ff, AF.Tanh, "ffb1"))
            nc.vector.memset(a_ff[0:1, 4, :], 1.0)
            feat = ap_.tile([P, T // P, H * E], BF16, name="feat", tag="feat", bufs=1)
            fwt = load_w("feataug", tag="vaug")
            aff_parts = [(kp, s, (lambda kp_, s_, tsl: a_ff[0:kp_, s_, tsl]))
                         for (kp, s) in (k_parts(a_ff, M) + [(1, 4)])]
            tok_matmul(feat, aff_parts, fwt, H * E)

            a_fw = ap_.tile([P, NSUB, T], BF16, name="afw", tag="hid4", bufs=2)
            linear_fm(load_w("fw1t"), h_cur, D, M, act_evict(a_fw, AF.Tanh, "fwb1"))

            wlg = load_w("wlogit")
            pool_w = sp.tile([8, T], BF16, name="pool_w", tag="poolw")
            accs = sp.tile([8, NCH], F32, name="accs", tag="accs")
            ml = sp.tile([8, TS], F32, name="ml", tag="ml", bufs=2)
            for ch in range(NCH):
                chs = slice(ch * TS, (ch + 1) * TS)
                ps_l = psB.tile([P, TS], F32, name="pslog", tag="psB", bufs=2)
                for s in range(NSUB):
                    nc.tensor.matmul(ps_l[0:8, :], wlg[:, s, 0:8], a_fw[:, s, chs],
                                     start=(s == 0), stop=(s == NSUB - 1))
                mlc = sp.tile([8, TS], F32, name="mlc", tag="ml", bufs=2)
                nc.vector.scalar_tensor_tensor(mlc[:, :], ps_l[0:8, :],
                                               bias_t[0:8, 0, BIAS_IDX["blogit"] : BIAS_IDX["blogit"] + 1],
                                               mask8_t[:, chs], op0=OP.add, op1=OP.add)
                nc.scalar.activation(pool_w[:, chs], mlc[:, :], AF.Exp,
                                     accum_out=accs[:, ch : ch + 1])
            psum8 = sp.tile([8, 1], F32, name="psum8", tag="psum8")
            nc.vector.tensor_add(psum8[:], accs[:, 0:1], accs[:, 1:2])
            pr8 = sp.tile([8, 1], F32, name="pr8", tag="pr8")
            nc.vector.reciprocal(pr8[:], psum8[:])
            nc.vector.tensor_scalar(pool_w[:], pool_w[:], pr8[:], None, op0=OP.mult)

            pool_wT = sp.tile([P, T // P, 8], BF16, name="pool_wT", tag="poolwT")
            for tb in range(T // P):
                ps_t = psC.tile([P, TS], BF16, name="pstp", tag="psC", bufs=2)
                nc.tensor.transpose(ps_t[0:P, 0:8], pool_w[:, tb * P : (tb + 1) * P],
                                    ident[0:8, 0:8])
                nc.vector.tensor_copy(pool_wT[:, tb, :], ps_t[0:P, 0:8])

            pooled = sp.tile([P, NSUB, 1], BF16, name="pooled", tag="pooled")
            for mb in range(NSUB):
                ps_p = psA.tile([P, TS], F32, name="pspool", tag="psA", bufs=4)
                for s in range(T // P):
                    nc.tensor.matmul(ps_p[:, 0:8], feat[:, s, mb * P : (mb + 1) * P],
                                     pool_wT[:, s, :], start=(s == 0), stop=(s == T // P - 1))
                for hh in (2 * mb, 2 * mb + 1):
                    pb = (hh % 2) * E
                    nc.vector.tensor_copy(pooled[pb : pb + E, mb, 0:1],
                                          ps_p[pb : pb + E, hh : hh + 1])

            wlast = load_w("wlast")
            for mb in range(NSUB):
                ps_o = psB.tile([P, TS], F32, name="psout", tag="psB", bufs=2)
                for s in range(NSUB):
                    nc.tensor.matmul(ps_o[:, 0:1], wlast[:, s, mb * P : (mb + 1) * P],
                                     pooled[:, s, 0:1], start=(s == 0), stop=(s == NSUB - 1))
                nc.scalar.activation(out_sb[:, mb, 0:1], ps_o[:, 0:1], AF.Identity,
                                     bias=bias_ap("blast", mb))
            nc.sync.dma_start(out=out_d[:].rearrange("(s p) -> p s", p=P), in_=out_sb[:, :, 0])

    nc.finalize()
    return nc


def layer_norm(nc, tc, sp, psB, psC, y, dst, gb, bias_ap, gname,
               ones_k, ones_m, ones_row, ap_):
    """dst = LN(y) over feature (partition-block) dim. y, dst: [P, NSUB, T] bf16."""
    for ch in range(NCH):
        chs = slice(ch * TS, (ch + 1) * TS)
        ps_sum = psB.tile([P, TS], F32, name="pssum", tag="psB", bufs=2)
        for s in range(NSUB):
            nc.tensor.matmul(ps_sum[0:1, :], ones_k[:], y[:, s, chs],
                             start=(s == 0), stop=(s == NSUB - 1))
        ysq = [None] * NSUB
        for s in range(NSUB):
            ysq[s] = sp.tile([P, TS], BF16, name=f"ysq{s}", tag="ysq", bufs=3)
            nc.vector.tensor_mul(ysq[s][:], y[:, s, chs], y[:, s, chs])
        ps_sq = psB.tile([P, TS], F32, name="pssq", tag="psB", bufs=2)
        for s in range(NSUB):
            nc.tensor.matmul(ps_sq[0:1, :], ones_k[:], ysq[s][:],
                             start=(s == 0), stop=(s == NSUB - 1))
        mean = sp.tile([1, TS], F32, name="mean", tag="vec", bufs=6)
        nc.vector.tensor_scalar(mean[:], ps_sum[0:1, :], 1.0 / D, None, op0=OP.mult)
        m2 = sp.tile([1, TS], F32, name="m2", tag="vec", bufs=6)
        nc.vector.tensor_mul(m2[:], mean[:], mean[:])
        var = sp.tile([1, TS], F32, name="var", tag="vec", bufs=6)
        nc.vector.scalar_tensor_tensor(var[:], ps_sq[0:1, :], 1.0 / D, m2[:],
                                       op0=OP.mult, op1=OP.subtract)
        eps_t = sp.tile([1, 1], F32, name="eps_t", tag="eps", bufs=1)
        nc.vector.memset(eps_t[:], 1e-5)
        sd = sp.tile([1, TS], F32, name="sd", tag="vec", bufs=6)
        nc.scalar.activation(sd[:], var[:], AF.Sqrt, bias=eps_t[:])
        rstd = sp.tile([1, TS], F32R, name="rstd", tag="vec", bufs=6)
        with nc.allow_low_precision("f32r rstd for broadcast matmul"):
            nc.vector.reciprocal(rstd[:], sd[:])
        c0 = sp.tile([1, TS], F32R, name="c0", tag="vec", bufs=6)
        nc.vector.scalar_tensor_tensor(c0[:], mean[:], -1.0, rstd[:],
                                       op0=OP.mult, op1=OP.mult)
        ps_abc = psC.tile([P, TS], F32, name="psabc", tag="psC", bufs=2)
        nc.tensor.matmul(ps_abc[:, :], ones_m[:], rstd[:], start=True, stop=True)
        for s in range(NSUB):
            ps_cbc = psB.tile([P, TS], F32, name="pscbc", tag="psB", bufs=2)
            g_row, b_row = gb
            nc.tensor.matmul(ps_cbc[:, :], g_row[0:1, s * P : (s + 1) * P], c0[:],
                             start=True, stop=False)
            nc.tensor.matmul(ps_cbc[:, :], b_row[0:1, s * P : (s + 1) * P], ones_row[:],
                             start=False, stop=True)
            s1 = sp.tile([P, TS], F32, name="s1", tag="s1", bufs=2)
            nc.vector.scalar_tensor_tensor(s1[:], y[:, s, chs], bias_ap(gname, s),
                                           ps_abc[:, :], op0=OP.mult, op1=OP.mult)
            nc.vector.tensor_tensor(dst[:, s, chs], s1[:], ps_cbc[:, :], op=OP.add)


# ---------------------------------------------------------------- entry points --

_CACHE = {}


def _ensure_shim():
    """Make antenv.axon_hooks importable so trace=True works under axon."""
    if "antenv.axon_hooks" in sys.modules:
        return
    try:
        import antenv
        from trn_agent_boot.trn_boot import _ntff_profile_via_ctypes
        hook = _ntff_profile_via_ctypes("/opt/axon/libaxon_pjrt.so")
        mod = types.ModuleType("antenv.axon_hooks")
        mod._hook = hook
        mod.get_axon_ntff_profile_hook = lambda: mod._hook
        mod.set_axon_ntff_profile_hook = lambda h: setattr(mod, "_hook", h)
        sys.modules["antenv.axon_hooks"] = mod
        antenv.axon_hooks = mod
    except Exception:
        pass


def kernel_run(x, lengths, params, trace=False):
    if "nc" not in _CACHE:
        _CACHE["nc"] = build_nc()
    nc = _CACHE["nc"]
    W = host_weights(params)
    x = np.asarray(x)
    lengths = np.asarray(lengths)
    in_maps = []
    for b in range(B):
        m = dict(W)
        m.update(host_core_inputs(x, lengths, b))
        in_maps.append(m)
    if trace:
        _ensure_shim()
    res = run_bass_kernel_spmd(nc, in_maps, core_ids=list(range(B)), trace=trace)
    out = np.stack([r["out"] for r in res.results]).astype(np.float32)
    return out, res


def kernel(x, lengths, params):
    out, _ = kernel_run(x, lengths, params)
    return out
